# revision 17
# baseline (speedup 1.0000x reference)
"""Trainium2 Bass kernel for nn_Attention (B=4, N=1024, DIM=1024, H=16).

Sharding: 8 cores = 4 batches x 2 query-halves of 512 rows each. No
collectives — each core recomputes its batch's K/V projections.

Matmuls run in bf16 (inputs pre-cast on host / in DVE copies) with fp32
PSUM accumulation.

Per-core pipeline:
  phase 1: KpT[d,k], Vp[k,(h,65)] (65th col = kmask01 -> softmax denom),
           QpT[d,q], Qp[q,d] projections.
  phase 2: per head pair: S^T[k,q] = Kh.Qh^T -> exp (no max subtraction;
           scores are tiny) -> (A.V | denom) via 65-wide lhsT -> PE
           transpose back to [q, 64] -> divide by denom -> O[q,d].
  phase 3: residual + LN1 -> transpose -> fc_o -> exact GELU + residual
           -> LN2 -> * qmask01 -> out.

Masking: masked-K rows are zeroed in Vp and in the denom column (exactly
the reference's post-softmax zeroing); masked-Q rows flow through as
finite garbage and are zeroed by the final qmask multiply.

Inputs are packed host-side so each phase's SBUF loads are a single DMA
(one completion semaphore -> at most one extra wait per matmul).
"""

import numpy as np
import ml_dtypes
from contextlib import ExitStack

import concourse.bass as bass
import concourse.bacc as bacc
import concourse.mybir as mybir
import concourse.tile as tile
from concourse.bass_utils import run_bass_kernel_spmd
from concourse.masks import make_identity

FP = mybir.dt.float32
BF = mybir.dt.bfloat16
AF = mybir.ActivationFunctionType
ALU = mybir.AluOpType

DIM = 1024
H = 16
DH = 64
B = 4
N = 1024          # keys per batch
NQ = 512          # queries per core
P = 128
NDT = DIM // P    # 8 feature tiles
NKT = N // P      # 8 key tiles
NQT = NQ // P     # 4 query tiles
EPS = 1e-5

_CACHED_NC = None


def _ln_apply(nc, pool, x_ap, out_ap, eps_sb, extra_scale=None):
    """LayerNorm (g=1, b=0) of x_ap [128, 1024] into out_ap."""
    stats = pool.tile([P, 2, 6], FP, tag="ln_stats", name="ln_stats")
    mv = pool.tile([P, 2], FP, tag="ln_mv", name="ln_mv")
    xg = x_ap.rearrange("p (s d) -> p s d", s=2)
    for s in range(2):
        nc.vector.bn_stats(out=stats[:, s, :], in_=xg[:, s, :])
    nc.vector.bn_aggr(out=mv, in_=stats)
    sd = pool.tile([P, 1], FP, tag="ln_sd", name="ln_sd")
    nc.scalar.activation(out=sd, in_=mv[:, 1:2], func=AF.Sqrt, bias=eps_sb)
    rstd = pool.tile([P, 1], FP, tag="ln_rstd", name="ln_rstd")
    nc.vector.reciprocal(out=rstd, in_=sd)
    if extra_scale is not None:
        nc.vector.tensor_mul(rstd, rstd, extra_scale)
    nc.vector.tensor_scalar(
        out=out_ap, in0=x_ap, scalar1=mv[:, 0:1], scalar2=rstd,
        op0=ALU.subtract, op1=ALU.mult,
    )


def build_nc():
    nc = bacc.Bacc(None, target_bir_lowering=False, debug=True)
    # packa: [P, 16, N] bf16 — j 0..7 = K.T row-tiles, 8..15 = (Wk.T/32) row-tiles
    packa = nc.declare_dram_parameter("packa", [P, 2 * NDT, N], BF, isOutput=False)
    packb = nc.declare_dram_parameter("packb", [P, 2 * NDT, N], BF, isOutput=False)
    # packc: [P, 8, 1536] — [:, j, 0:512] = Q.T row-tiles, [:, j, 512:1536] = Wq.T
    packc = nc.declare_dram_parameter("packc", [P, NDT, NQ + DIM], BF, isOutput=False)
    wo = nc.declare_dram_parameter("wo", [P, NDT, DIM], BF, isOutput=False)
    # maskd: [P, 12] f32 — cols 0..7 = kmask01 tiles, 8..11 = qmask01 tiles
    maskd = nc.declare_dram_parameter("maskd", [P, NKT + NQT], FP, isOutput=False)
    out = nc.declare_dram_parameter("out", [NQ, DIM], FP, isOutput=True)

    with ExitStack() as ctx:
        tc = ctx.enter_context(tile.TileContext(nc))
        persist = ctx.enter_context(tc.tile_pool(name="persist", bufs=1))

        KpT = [persist.tile([P, N], BF, tag=f"kpt{i}", name=f"kpt{i}") for i in range(NDT)]
        Vp = [persist.tile([P, H, DH + 1], BF, tag=f"vp{i}", name=f"vp{i}") for i in range(NKT)]
        Qp = [persist.tile([P, DIM], FP, tag=f"qp{t}", name=f"qp{t}") for t in range(NQT)]
        O = [persist.tile([P, DIM], FP, tag=f"o{t}", name=f"o{t}") for t in range(NQT)]
        identb = persist.tile([P, P], BF, tag="identb", name="identb")
        make_identity(nc, identb)
        eps_sb = persist.tile([P, 1], FP, tag="eps", name="eps_sb")
        nc.vector.memset(eps_sb, EPS)
        mask_sb = persist.tile([P, NKT + NQT], FP, tag="maskd", name="mask_sb")
        pa = persist.tile([P, 2 * NDT, N], BF, tag="pa", name="pa_sb")
        pb = persist.tile([P, 2 * NDT, N], BF, tag="pb", name="pb_sb")
        pc = persist.tile([P, NDT, NQ + DIM], BF, tag="pc", name="pc_sb")
        wo_sb3 = persist.tile([P, NDT, DIM], BF, tag="wosb", name="wosb3")
        nc.sync.dma_start(out=mask_sb, in_=maskd[:, :])
        km_sb = mask_sb[:, 0:NKT]
        qm_sb = mask_sb[:, NKT:NKT + NQT]

        # ---------- phase 1a: KpT[dout, k] ----------
        with tc.tile_pool(name="p1ap", bufs=4, space="PSUM") as p1ap:
            nc.sync.dma_start(out=pa, in_=packa[:, :, :])
            for i in range(NDT):
                for c in range(2):
                    ps = p1ap.tile([P, 512], FP, tag="ps", name="ps1a")
                    for j in range(NDT):
                        nc.tensor.matmul(ps, pa[:, NDT + j, i * P:(i + 1) * P],
                                         pa[:, j, c * 512:(c + 1) * 512],
                                         start=(j == 0), stop=(j == NDT - 1))
                    nc.vector.tensor_copy(KpT[i][:, c * 512:(c + 1) * 512], ps)

        # ---------- phase 1b: Vp[k, dout], masked, 65-col head layout ----------
        with tc.tile_pool(name="p1bp", bufs=4, space="PSUM") as p1bp:
            nc.sync.dma_start(out=pb, in_=packb[:, :, :])
            for c in range(2):
                for i in range(NKT):
                    ps = p1bp.tile([P, 512], FP, tag="ps", name="ps1b")
                    for j in range(NDT):
                        nc.tensor.matmul(ps, pb[:, j, i * P:(i + 1) * P],
                                         pb[:, NDT + j, c * 512:(c + 1) * 512],
                                         start=(j == 0), stop=(j == NDT - 1))
                    nc.vector.tensor_scalar_mul(
                        out=Vp[i][:, 8 * c:8 * c + 8, 0:DH],
                        in0=ps.rearrange("p (h d) -> p h d", h=8),
                        scalar1=km_sb[:, i:i + 1])
            for i in range(NKT):
                nc.vector.tensor_copy(Vp[i][:, :, DH:DH + 1],
                                      km_sb[:, i:i + 1].to_broadcast((P, H, 1)))

        # ---------- phase 1c: QpT[dout, q] and Qp[q, dout] ----------
        midctx = ExitStack()
        midpool = midctx.enter_context(tc.tile_pool(name="mid", bufs=1))
        QpT = [midpool.tile([P, NQ], BF, tag=f"qpt{i}", name=f"qpt{i}") for i in range(NDT)]
        with tc.tile_pool(name="p1cp", bufs=4, space="PSUM") as p1cp:
            nc.sync.dma_start(out=pc, in_=packc[:, :, :])
            qt_sb = [pc[:, j, 0:NQ] for j in range(NDT)]
            wq_sb = [pc[:, j, NQ:NQ + DIM] for j in range(NDT)]
            for i in range(NDT):
                ps = p1cp.tile([P, 512], FP, tag="ps", name="ps1c")
                for j in range(NDT):
                    nc.tensor.matmul(ps, wq_sb[j][:, i * P:(i + 1) * P], qt_sb[j],
                                     start=(j == 0), stop=(j == NDT - 1))
                nc.vector.tensor_copy(QpT[i], ps)
            for t in range(NQT):
                for c in range(2):
                    ps = p1cp.tile([P, 512], FP, tag="ps", name="ps1c2")
                    for j in range(NDT):
                        nc.tensor.matmul(ps, qt_sb[j][:, t * P:(t + 1) * P],
                                         wq_sb[j][:, c * 512:(c + 1) * 512],
                                         start=(j == 0), stop=(j == NDT - 1))
                    nc.vector.tensor_copy(Qp[t][:, c * 512:(c + 1) * 512], ps)

        # ---------- phase 2: attention, head pairs ----------
        with tc.tile_pool(name="p2es", bufs=2) as p2es, \
             tc.tile_pool(name="p2sb", bufs=3) as p2sb, \
             tc.tile_pool(name="p2sm", bufs=8) as p2sm, \
             tc.tile_pool(name="sps", bufs=4, space="PSUM") as sps, \
             tc.tile_pool(name="avs", bufs=2, space="PSUM") as avs, \
             tc.tile_pool(name="tps", bufs=2, space="PSUM") as tps:
            for hp in range(H // 2):
                avps = [avs.tile([DH + 1, NQ], FP, tag="av", name=f"av{hp}_{s}")
                        for s in range(2)]
                for j in range(NKT):
                    for s in range(2):
                        h = 2 * hp + s
                        po = DH * s
                        sp = sps.tile([P, NQ], FP, tag="sp", name=f"sp{hp}_{j}_{s}")
                        nc.tensor.matmul(
                            sp,
                            KpT[hp][po:po + DH, j * P:(j + 1) * P],
                            QpT[hp][po:po + DH, :],
                            start=True, stop=True)
                        es = p2es.tile([P, NQ], BF, tag=f"es{j}", name=f"es{hp}_{j}_{s}")
                        nc.scalar.activation(out=es, in_=sp, func=AF.Exp)
                        nc.tensor.matmul(avps[s], Vp[j][:, h, :], es,
                                         start=(j == 0), stop=(j == NKT - 1))
                for s in range(2):
                    h = 2 * hp + s
                    avsb = p2sb.tile([DH + 1, NQ], BF, tag="avsb", name=f"avsb{hp}_{s}")
                    nc.vector.tensor_copy(avsb, avps[s])
                    for t in range(NQT):
                        tp = tps.tile([P, DH + 1], BF, tag="tp", name=f"tp{hp}_{s}_{t}")
                        nc.tensor.transpose(tp, avsb[:, t * P:(t + 1) * P],
                                            identb[0:DH + 1, 0:DH + 1])
                        dr = p2sm.tile([P, 1], FP, tag="dr", name=f"dr{hp}_{s}_{t}")
                        nc.vector.reciprocal(out=dr, in_=tp[:, DH:DH + 1])
                        nc.vector.tensor_scalar_mul(
                            out=O[t][:, h * DH:(h + 1) * DH],
                            in0=tp[:, 0:DH], scalar1=dr)
        midctx.close()

        # ---------- phase 3: residual + LN1 + fc_o + GELU + LN2 ----------
        with tc.tile_pool(name="p3", bufs=1) as p3, \
             tc.tile_pool(name="p3s", bufs=2) as p3s, \
             tc.tile_pool(name="p3p", bufs=4, space="PSUM") as p3p, \
             tc.tile_pool(name="tps3", bufs=4, space="PSUM") as tps3:
            nc.sync.dma_start(out=wo_sb3, in_=wo[:, :, :])
            wo_sb = [wo_sb3[:, j] for j in range(NDT)]
            O1 = [p3.tile([P, DIM], BF, tag=f"o1_{t}", name=f"o1_{t}") for t in range(NQT)]
            OT = [p3.tile([P, NQ], BF, tag=f"ot{i}", name=f"ot{i}") for i in range(NDT)]
            for t in range(NQT):
                r1 = p3s.tile([P, DIM], FP, tag="r1", name=f"r1_{t}")
                nc.vector.tensor_add(r1, Qp[t], O[t])
                _ln_apply(nc, p3s, r1, O1[t], eps_sb)
                for i in range(NDT):
                    tp = tps3.tile([P, P], BF, tag="tp3", name=f"tp3_{t}_{i}")
                    nc.tensor.transpose(tp, O1[t][:, i * P:(i + 1) * P], identb)
                    nc.vector.tensor_copy(OT[i][:, t * P:(t + 1) * P], tp)
            for t in range(NQT):
                g = p3s.tile([P, DIM], FP, tag="g", name=f"g_{t}")
                for c in range(2):
                    ps = p3p.tile([P, 512], FP, tag="hps", name=f"hps_{t}_{c}")
                    for i in range(NDT):
                        nc.tensor.matmul(ps, OT[i][:, t * P:(t + 1) * P],
                                         wo_sb[i][:, c * 512:(c + 1) * 512],
                                         start=(i == 0), stop=(i == NDT - 1))
                    nc.scalar.activation(out=g[:, c * 512:(c + 1) * 512], in_=ps, func=AF.Gelu)
                r2 = p3s.tile([P, DIM], FP, tag="r1", name=f"r2_{t}")
                nc.vector.tensor_add(r2, O1[t], g)
                fin = p3s.tile([P, DIM], FP, tag="g", name=f"fin_{t}")
                _ln_apply(nc, p3s, r2, fin, eps_sb, extra_scale=qm_sb[:, t:t + 1])
                nc.sync.dma_start(out=out[t * P:(t + 1) * P, :], in_=fin)

    nc.compile()
    return nc


def _get_nc():
    global _CACHED_NC
    if _CACHED_NC is None:
        _CACHED_NC = build_nc()
    return _CACHED_NC


def _pack_rows(mats):
    """[t*128, n] row-major mats -> one [128, sum_t, n] array (j-tile minor)."""
    blocks = []
    for m in mats:
        r, n = m.shape
        blocks.append(m.reshape(r // P, P, n).transpose(1, 0, 2))
    return np.concatenate(blocks, axis=1)


def _make_in_maps(inputs):
    Q, K, V = inputs["Q"], inputs["K"], inputs["V"]
    mask_Q, mask_K = inputs["mask_Q"], inputs["mask_K"]
    bf = ml_dtypes.bfloat16
    sc = 1.0 / np.sqrt(np.float32(DIM))
    wqT = np.ascontiguousarray(inputs["Wq"].T)
    wkT = np.ascontiguousarray(inputs["Wk"].T) * sc
    wvT = np.ascontiguousarray(inputs["Wv"].T)
    woT = np.ascontiguousarray(_pack_rows([np.ascontiguousarray(inputs["Wo"].T)])).astype(bf)
    in_maps = []
    for c in range(8):
        b, q0 = c // 2, (c % 2) * NQ
        kt = np.ascontiguousarray(K[b].T)
        vt = np.ascontiguousarray(V[b].T)
        qt = np.ascontiguousarray(Q[b, q0:q0 + NQ, :].T)
        packa = np.ascontiguousarray(_pack_rows([kt, wkT])).astype(bf)
        packb = np.ascontiguousarray(_pack_rows([vt, wvT])).astype(bf)
        qt_j = qt.reshape(NDT, P, NQ).transpose(1, 0, 2)
        wq_j = wqT.reshape(NDT, P, DIM).transpose(1, 0, 2)
        packc = np.ascontiguousarray(np.concatenate([qt_j, wq_j], axis=2)).astype(bf)
        km01 = np.where(mask_K[b], 0.0, 1.0).astype(np.float32)
        qm01 = np.where(mask_Q[b, q0:q0 + NQ], 0.0, 1.0).astype(np.float32)
        maskd = np.concatenate([km01.reshape(NKT, P).T,
                                qm01.reshape(NQT, P).T], axis=1)
        in_maps.append({
            "packa": packa, "packb": packb, "packc": packc, "wo": woT,
            "maskd": np.ascontiguousarray(maskd),
        })
    return in_maps


def _assemble(results):
    out = np.empty((B, 1024, DIM), np.float32)
    for c in range(8):
        b, q0 = c // 2, (c % 2) * NQ
        out[b, q0:q0 + NQ, :] = results[c]["out"]
    return out


def kernel(**inputs):
    nc = _get_nc()
    res = run_bass_kernel_spmd(nc, _make_in_maps(inputs), core_ids=list(range(8)))
    return _assemble(res.results)


def kernel_profiled(inputs, **kw):
    nc = _get_nc()
    res = run_bass_kernel_spmd(nc, _make_in_maps(inputs),
                               core_ids=list(range(8)), trace=True, **kw)
    return _assemble(res.results), res


# revision 43
# speedup vs baseline: 20205.1339x; 20205.1339x over previous
"""Trainium2 Bass kernel for nn_Attention (B=4, N=1024, DIM=1024, H=16).

Sharding: 8 cores = 4 batches x 2 query-halves of 512 rows each. No
collectives — each core recomputes its batch's K/V projections.

Matmuls run in bf16 (inputs pre-cast on host / in DVE copies) with fp32
PSUM accumulation.

Per-core pipeline:
  phase 1: KpT[d,k], Vp[k,(h,65)] (65th col = kmask01 -> softmax denom),
           QpT[d,q], Qp[q,d] projections.
  phase 2: per head pair: S^T[k,q] = Kh.Qh^T -> exp (no max subtraction;
           scores are tiny) -> (A.V | denom) via 65-wide lhsT -> PE
           transpose back to [q, 64] -> divide by denom -> O[q,d].
  phase 3: residual + LN1 -> transpose -> fc_o -> exact GELU + residual
           -> LN2 -> * qmask01 -> out.

Masking: masked-K rows are zeroed in Vp and in the denom column (exactly
the reference's post-softmax zeroing); masked-Q rows flow through as
finite garbage and are zeroed by the final qmask multiply.

Inputs are packed host-side so each phase's SBUF loads are a single DMA
(one completion semaphore -> at most one extra wait per matmul).
"""

import numpy as np
import ml_dtypes
from contextlib import ExitStack

import concourse.bass as bass
import concourse.bacc as bacc
import concourse.mybir as mybir
import concourse.tile as tile
from concourse.bass_utils import run_bass_kernel_spmd
from concourse.masks import make_identity

FP = mybir.dt.float32
BF = mybir.dt.bfloat16
AF = mybir.ActivationFunctionType
ALU = mybir.AluOpType

DIM = 1024
H = 16
DH = 64
B = 4
N = 1024          # keys per batch
NQ = 512          # queries per core
P = 128
NDT = DIM // P    # 8 feature tiles
NKT = N // P      # 8 key tiles
NQT = NQ // P     # 4 query tiles
EPS = 1e-5

_CACHED_NC = None


def _ln_apply(nc, pool, x_ap, out_ap, eps_sb, extra_scale=None):
    """LayerNorm (g=1, b=0) of x_ap [128, 1024] into out_ap."""
    stats = pool.tile([P, 2, 6], FP, tag="ln_stats", name="ln_stats", bufs=4)
    mv = pool.tile([P, 2], FP, tag="ln_mv", name="ln_mv", bufs=4)
    xg = x_ap.rearrange("p (s d) -> p s d", s=2)
    for s in range(2):
        nc.vector.bn_stats(out=stats[:, s, :], in_=xg[:, s, :])
    nc.vector.bn_aggr(out=mv, in_=stats)
    sd = pool.tile([P, 1], FP, tag="ln_sd", name="ln_sd", bufs=4)
    nc.scalar.activation(out=sd, in_=mv[:, 1:2], func=AF.Sqrt, bias=eps_sb)
    rstd = pool.tile([P, 1], FP, tag="ln_rstd", name="ln_rstd", bufs=4)
    nc.vector.reciprocal(out=rstd, in_=sd)
    if extra_scale is not None:
        nc.vector.tensor_mul(rstd, rstd, extra_scale)
    nc.vector.tensor_scalar(
        out=out_ap, in0=x_ap, scalar1=mv[:, 0:1], scalar2=rstd,
        op0=ALU.subtract, op1=ALU.mult,
    )


def build_nc(phases=3):
    nc = bacc.Bacc(None, target_bir_lowering=False, debug=True)
    # packa: [P, 16, N] bf16 — j 0..7 = K.T row-tiles, 8..15 = (Wk.T/32) row-tiles
    packa = nc.declare_dram_parameter("packa", [P, 2 * NDT, N], BF, isOutput=False)
    packb = nc.declare_dram_parameter("packb", [P, 2 * NDT, N], BF, isOutput=False)
    # packc: [P, 8, 1536] — [:, j, 0:512] = Q.T row-tiles, [:, j, 512:1536] = Wq.T
    packc = nc.declare_dram_parameter("packc", [P, NDT, NQ + DIM], BF, isOutput=False)
    wo = nc.declare_dram_parameter("wo", [P, NDT, DIM], BF, isOutput=False)
    # maskd: [P, 12] f32 — cols 0..7 = kmask01 tiles, 8..11 = qmask01 tiles
    maskd = nc.declare_dram_parameter("maskd", [P, NKT + NQT], FP, isOutput=False)
    out = nc.declare_dram_parameter("out", [NQ, DIM], FP, isOutput=True)

    with ExitStack() as ctx:
        tc = ctx.enter_context(tile.TileContext(nc))
        persist = ctx.enter_context(tc.tile_pool(name="persist", bufs=1))

        KpT = [persist.tile([P, N], BF, tag=f"kpt{i}", name=f"kpt{i}") for i in range(NDT)]
        Vp = [persist.tile([P, H, DH + 1], BF, tag=f"vp{i}", name=f"vp{i}") for i in range(NKT)]
        Qp = [persist.tile([P, DIM], BF, tag=f"qp{t}", name=f"qp{t}") for t in range(NQT)]
        Ob = persist.tile([P, NQT, DIM], FP, tag="ob", name="ob")
        identb = persist.tile([P, P], BF, tag="identb", name="identb")
        make_identity(nc, identb)
        eps_sb = persist.tile([P, 1], FP, tag="eps", name="eps_sb")
        nc.vector.memset(eps_sb, EPS)
        mask_sb = persist.tile([P, NKT + NQT], FP, tag="maskd", name="mask_sb")
        pa = persist.tile([P, 2 * NDT, N], BF, tag="pa", name="pa_sb")
        pb = persist.tile([P, 2 * NDT, N], BF, tag="pb", name="pb_sb")
        pc = persist.tile([P, NDT, NQ + DIM], BF, tag="pc", name="pc_sb")
        wo_sb3 = persist.tile([P, NDT, DIM], BF, tag="wosb", name="wosb3")
        nc.sync.dma_start(out=mask_sb, in_=maskd[:, :])
        km_sb = mask_sb[:, 0:NKT]
        qm_sb = mask_sb[:, NKT:NKT + NQT]

        # ---------- phase 1a: KpT[dout, k] ----------
        with tc.tile_pool(name="p1ap", bufs=4, space="PSUM") as p1ap:
            pa_d = packa[:, :, :].rearrange("p (x j) n -> p j x n", x=2)
            pa_v = pa.rearrange("p (x j) n -> p j x n", x=2)
            for j in range(NDT):
                nc.sync.dma_start(out=pa_v[:, j], in_=pa_d[:, j])
            for i in range(NDT):
                for c in range(2):
                    ps = p1ap.tile([P, 512], FP, tag="ps", name="ps1a")
                    for j in range(NDT):
                        nc.tensor.matmul(ps, pa[:, NDT + j, i * P:(i + 1) * P],
                                         pa[:, j, c * 512:(c + 1) * 512],
                                         start=(j == 0), stop=(j == NDT - 1))
                    nc.vector.tensor_copy(KpT[i][:, c * 512:(c + 1) * 512], ps)

        # ---------- phase 1c: QpT[dout, q] and Qp[q, dout] ----------
        midctx = ExitStack()
        midpool = midctx.enter_context(tc.tile_pool(name="mid", bufs=1))
        QpT = [midpool.tile([P, NQ], BF, tag=f"qpt{i}", name=f"qpt{i}") for i in range(NDT)]
        with tc.tile_pool(name="p1cp", bufs=2, space="PSUM") as p1cp:
            for j in range(NDT):
                nc.sync.dma_start(out=pc[:, j], in_=packc[:, j, :])
            qt_sb = [pc[:, j, 0:NQ] for j in range(NDT)]
            wq_sb = [pc[:, j, NQ:NQ + DIM] for j in range(NDT)]
            for i in range(NDT):
                ps = p1cp.tile([P, 512], FP, tag="ps", name="ps1c")
                for j in range(NDT):
                    nc.tensor.matmul(ps, wq_sb[j][:, i * P:(i + 1) * P], qt_sb[j],
                                     start=(j == 0), stop=(j == NDT - 1))
                nc.vector.tensor_copy(QpT[i], ps)
            for t in range(NQT):
                for i in range(NDT):
                    tq = p1cp.tile([P, P], BF, tag="tq", name=f"tq_{t}_{i}")
                    nc.tensor.transpose(tq, QpT[i][:, t * P:(t + 1) * P], identb)
                    nc.vector.tensor_copy(Qp[t][:, i * P:(i + 1) * P], tq)
            # head pair 0: scores+exp early so ACT overlaps phase 1b
            with tc.tile_pool(name="spre", bufs=2, space="PSUM") as spre:
                es_pre = []
                for j in range(NKT):
                    sp = spre.tile([P, 2, NQ], FP, tag="spp", name=f"spp{j}")
                    for s in range(2):
                        po = DH * s
                        nc.tensor.matmul(
                            sp[:, s, :],
                            KpT[0][po:po + DH, j * P:(j + 1) * P],
                            QpT[0][po:po + DH, :],
                            start=True, stop=True)
                    es = midpool.tile([P, 2, NQ], BF, tag=f"esp{j}", name=f"esp{j}")
                    nc.scalar.activation(out=es, in_=sp, func=AF.Exp)
                    es_pre.append(es)

        if phases < 2:
            midctx.close()
            return _finish(nc)
        # ---------- phase 1b: Vp[k, dout], masked, 65-col head layout ----------
        with tc.tile_pool(name="p1bp", bufs=4, space="PSUM") as p1bp:
            pb_d = packb[:, :, :].rearrange("p (x j) n -> p j x n", x=2)
            pb_v = pb.rearrange("p (x j) n -> p j x n", x=2)
            for j in range(NDT):
                nc.sync.dma_start(out=pb_v[:, j], in_=pb_d[:, j])
            for c in range(2):
                for i in range(NKT):
                    ps = p1bp.tile([P, 512], FP, tag="ps", name="ps1b")
                    for j in range(NDT):
                        nc.tensor.matmul(ps, pb[:, j, i * P:(i + 1) * P],
                                         pb[:, NDT + j, c * 512:(c + 1) * 512],
                                         start=(j == 0), stop=(j == NDT - 1))
                    nc.vector.tensor_scalar_mul(
                        out=Vp[i][:, 8 * c:8 * c + 8, 0:DH],
                        in0=ps.rearrange("p (h d) -> p h d", h=8),
                        scalar1=km_sb[:, i:i + 1])
            for i in range(NKT):
                nc.vector.tensor_copy(Vp[i][:, :, DH:DH + 1],
                                      km_sb[:, i:i + 1].to_broadcast((P, H, 1)))

        # ---------- phase 2: attention, head pairs ----------
        with tc.tile_pool(name="p2es", bufs=1) as p2es, \
             tc.tile_pool(name="p2sb", bufs=2) as p2sb, \
             tc.tile_pool(name="p2sm", bufs=8) as p2sm, \
             tc.tile_pool(name="sps", bufs=2, space="PSUM") as sps:
            avtp = ExitStack()
            avs = avtp.enter_context(tc.tile_pool(name="avs", bufs=3, space="PSUM"))
            tps = avtp.enter_context(tc.tile_pool(name="tps", bufs=1, space="PSUM"))
            for hp in range(H // 2):
                avps = [avs.tile([DH + 1, NQ], FP, tag="av", name=f"av{hp}_{s}")
                        for s in range(2)]
                for j in range(NKT):
                    sp = sps.tile([P, 2, NQ], FP, tag="sp", name=f"sp{hp}_{j}")
                    for s in range(2):
                        po = DH * s
                        nc.tensor.matmul(
                            sp[:, s, :],
                            KpT[hp][po:po + DH, j * P:(j + 1) * P],
                            QpT[hp][po:po + DH, :],
                            start=True, stop=True)
                    es = p2es.tile([P, 2, NQ], BF, tag=f"es{j}", name=f"es{hp}_{j}")
                    nc.scalar.activation(out=es, in_=sp, func=AF.Exp)
                    for s in range(2):
                        h = 2 * hp + s
                        nc.tensor.matmul(avps[s], Vp[j][:, h, :], es[:, s, :],
                                         start=(j == 0), stop=(j == NKT - 1))
                for s in range(2):
                    h = 2 * hp + s
                    avsb = p2sb.tile([DH + 1, NQ], BF, tag="avsb", name=f"avsb{hp}_{s}")
                    nc.vector.tensor_copy(avsb, avps[s])
                    tpg = tps.tile([P, NQT, DH + 2], BF, tag="tp", name=f"tp{hp}_{s}")
                    for t in range(NQT):
                        nc.tensor.matmul(tpg[:, t, 0:DH + 1], avsb[:, t * P:(t + 1) * P],
                                         identb[0:DH + 1, 0:DH + 1],
                                         is_transpose=True,
                                         start=(t == 0), stop=(t == NQT - 1))
                    osb = p2sm.tile([P, NQT, DH + 2], BF, tag="osb", name=f"osb{hp}_{s}")
                    nc.vector.tensor_copy(osb[:, :, 0:DH + 1], tpg[:, :, 0:DH + 1])
                    dr = p2sm.tile([P, NQT, 1], FP, tag="dr", name=f"dr{hp}_{s}")
                    nc.vector.reciprocal(out=dr, in_=osb[:, :, DH:DH + 1])
                    nc.vector.tensor_mul(
                        Ob[:, :, h * DH:(h + 1) * DH],
                        osb[:, :, 0:DH],
                        dr.to_broadcast((P, NQT, DH)))
            avtp.close()
        midctx.close()
        if phases < 3:
            return _finish(nc)

        # ---------- phase 3: residual + LN1 + fc_o + GELU + LN2 ----------
        with tc.tile_pool(name="p3", bufs=1) as p3, \
             tc.tile_pool(name="p3s", bufs=1) as p3s, \
             tc.tile_pool(name="p3p", bufs=4, space="PSUM") as p3p, \
             tc.tile_pool(name="tps3", bufs=4, space="PSUM") as tps3:
            nc.sync.dma_start(out=wo_sb3, in_=wo[:, :, :])
            wo_sb = [wo_sb3[:, j] for j in range(NDT)]
            O1 = [p3.tile([P, DIM], BF, tag=f"o1_{t}", name=f"o1_{t}") for t in range(NQT)]
            OTb = p3.tile([P, NDT, NQ], BF, tag="otb", name="otb")
            OT = [OTb[:, i] for i in range(NDT)]
            for t in range(NQT):
                r1 = p3s.tile([P, DIM], FP, tag="r1", name=f"r1_{t}", bufs=3)
                nc.vector.tensor_add(r1, Qp[t], Ob[:, t])
                _ln_apply(nc, p3s, r1, O1[t], eps_sb)
                tp = tps3.tile([P, NDT, P], BF, tag="tp3", name=f"tp3_{t}")
                for i in range(NDT):
                    nc.tensor.matmul(tp[:, i, :], O1[t][:, i * P:(i + 1) * P], identb,
                                     is_transpose=True,
                                     start=(i == 0), stop=(i == NDT - 1))
                nc.vector.tensor_copy(OTb[:, :, t * P:(t + 1) * P], tp)
            for t in range(NQT):
                g = p3s.tile([P, DIM], FP, tag="g", name=f"g_{t}", bufs=2)
                r2 = p3s.tile([P, DIM], FP, tag="r1", name=f"r2_{t}", bufs=3)
                for c in range(2):
                    ps = p3p.tile([P, 512], FP, tag="hps", name=f"hps_{t}_{c}")
                    for i in range(NDT):
                        nc.tensor.matmul(ps, OT[i][:, t * P:(t + 1) * P],
                                         wo_sb[i][:, c * 512:(c + 1) * 512],
                                         start=(i == 0), stop=(i == NDT - 1))
                    nc.scalar.activation(out=g[:, c * 512:(c + 1) * 512], in_=ps, func=AF.Gelu)
                    nc.vector.tensor_add(r2[:, c * 512:(c + 1) * 512], O1[t][:, c * 512:(c + 1) * 512],
                                         g[:, c * 512:(c + 1) * 512])
                fin = p3s.tile([P, DIM], FP, tag="g", name=f"fin_{t}", bufs=2)
                _ln_apply(nc, p3s, r2, fin, eps_sb, extra_scale=qm_sb[:, t:t + 1])
                nc.sync.dma_start(out=out[t * P:(t + 1) * P, :], in_=fin)

    return _finish(nc)


def _finish(nc):
    nc.compile()
    return nc


def _get_nc():
    global _CACHED_NC
    if _CACHED_NC is None:
        _CACHED_NC = build_nc()
    return _CACHED_NC


def _pack_rows(mats):
    """[t*128, n] row-major mats -> one [128, sum_t, n] array (j-tile minor)."""
    blocks = []
    for m in mats:
        r, n = m.shape
        blocks.append(m.reshape(r // P, P, n).transpose(1, 0, 2))
    return np.concatenate(blocks, axis=1)


def _make_in_maps(inputs):
    Q, K, V = inputs["Q"], inputs["K"], inputs["V"]
    mask_Q, mask_K = inputs["mask_Q"], inputs["mask_K"]
    bf = ml_dtypes.bfloat16
    sc = 1.0 / np.sqrt(np.float32(DIM))
    wqT = np.ascontiguousarray(inputs["Wq"].T)
    wkT = np.ascontiguousarray(inputs["Wk"].T) * sc
    wvT = np.ascontiguousarray(inputs["Wv"].T)
    woT = np.ascontiguousarray(_pack_rows([np.ascontiguousarray(inputs["Wo"].T)])).astype(bf)
    in_maps = []
    for c in range(8):
        b, q0 = c // 2, (c % 2) * NQ
        kt = np.ascontiguousarray(K[b].T)
        vt = np.ascontiguousarray(V[b].T)
        qt = np.ascontiguousarray(Q[b, q0:q0 + NQ, :].T)
        packa = np.ascontiguousarray(_pack_rows([kt, wkT])).astype(bf)
        packb = np.ascontiguousarray(_pack_rows([vt, wvT])).astype(bf)
        qt_j = qt.reshape(NDT, P, NQ).transpose(1, 0, 2)
        wq_j = wqT.reshape(NDT, P, DIM).transpose(1, 0, 2)
        packc = np.ascontiguousarray(np.concatenate([qt_j, wq_j], axis=2)).astype(bf)
        km01 = np.where(mask_K[b], 0.0, 1.0).astype(np.float32)
        qm01 = np.where(mask_Q[b, q0:q0 + NQ], 0.0, 1.0).astype(np.float32)
        maskd = np.concatenate([km01.reshape(NKT, P).T,
                                qm01.reshape(NQT, P).T], axis=1)
        in_maps.append({
            "packa": packa, "packb": packb, "packc": packc, "wo": woT,
            "maskd": np.ascontiguousarray(maskd),
        })
    return in_maps


def _assemble(results):
    out = np.empty((B, 1024, DIM), np.float32)
    for c in range(8):
        b, q0 = c // 2, (c % 2) * NQ
        out[b, q0:q0 + NQ, :] = results[c]["out"]
    return out


def kernel(**inputs):
    nc = _get_nc()
    res = run_bass_kernel_spmd(nc, _make_in_maps(inputs), core_ids=list(range(8)))
    return _assemble(res.results)


def kernel_profiled(inputs, **kw):
    nc = _get_nc()
    res = run_bass_kernel_spmd(nc, _make_in_maps(inputs),
                               core_ids=list(range(8)), trace=True, **kw)
    return _assemble(res.results), res
